# revision 1
# baseline (speedup 1.0000x reference)
"""Trainium2 Bass kernel: PaLM-style parallel attention + FF transformer block.

Tensor-parallel over 8 NeuronCores: each core owns 2 heads (128 q/k/v cols of
W_in), 512 FF cols, and the matching 640 rows of W_out.  Each core computes a
full-shape partial output; the host sums the 8 partials (row-parallel W_out).

Per-core dataflow (all matmuls in float32r, K-contractions on partitions):
  LN stats (bn_stats, token-major) -> xn -> PE-transpose -> xnT [D,T]
  hT = W_slice^T @ xnT   (q,k rope-fused eviction; v re-transposed token-major
                          with a ones column for softmax sums; ff -> gelu)
  per (i-chunk, head): ST[j,i] = kT^T q, PT = exp(0.125*ST) * causal_mask,
                       OT[0:64]=V^T PT accum, OT[64:128]=col-sums (ones cols)
  out_partial = [oT; ffgT]^T @ Wo_slice   (written full-shape, host-summed)
"""

import numpy as np

HEADS = 16
HEAD_DIM = 64
HIDDEN = 1024
EXPF = 4
B = 2
L = 2048
NCORES = 8
HPC = HEADS // NCORES            # heads per core = 2
QS = HPC * HEAD_DIM              # per-core q/k/v width = 128
FFS = EXPF * HIDDEN // NCORES    # per-core ff width = 512
WSL = 3 * QS + FFS               # per-core W_in slice width = 896
KOUT = HIDDEN // 128             # 8 k-subtiles for hidden contraction
WOK = (QS + FFS) // 128          # 5 k-subtiles for out-proj contraction
LN_EPS = 1e-5

LAST_RESULTS = None  # BassKernelResults of the most recent HW run (for test.py)


# ----------------------------------------------------------------------------
# program builder
# ----------------------------------------------------------------------------

def build_program(b=B, l=L, use_fp32r=True, debug=False, sim_gelu=False,
                  opts=None):
    import concourse.bass as bass
    import concourse.mybir as mybir
    import concourse.tile as tile
    from concourse import bacc

    T = b * l
    NT = l // 128      # 128-token tiles per batch
    NC = l // 512      # 512-token chunks per batch
    f32 = mybir.dt.float32
    mmdt = mybir.dt.float32r if use_fp32r else mybir.dt.float32
    bf16 = mybir.dt.bfloat16
    AF = mybir.ActivationFunctionType
    OP = mybir.AluOpType

    opts = {"gp_mask": False, "act_copies": True, "approx_recip": False,
            "bf16_inproj": False,
            **(opts or {})}
    nc = bacc.Bacc("TRN2", target_bir_lowering=False, debug=debug)

    x_d = nc.declare_dram_parameter("x", [T, HIDDEN], f32, isOutput=False)
    w_dt = bf16 if opts["bf16_inproj"] else mmdt
    w_d = nc.declare_dram_parameter("w_in", [HIDDEN, WSL], w_dt, isOutput=False)
    wo_d = nc.declare_dram_parameter("w_out", [QS + FFS, HIDDEN], mmdt, isOutput=False)
    hb_d = nc.declare_dram_parameter("h_bias", [WSL], f32, isOutput=False)
    cos_d = nc.declare_dram_parameter("cos_t", [128, l], f32, isOutput=False)
    sinm_d = nc.declare_dram_parameter("sinm_t", [128, l], f32, isOutput=False)
    mask_d = nc.declare_dram_parameter("mask_t", [128, 896], f32, isOutput=False)
    id_d = nc.declare_dram_parameter("ident", [128, 128], f32, isOutput=False)
    out_d = nc.declare_dram_parameter("out", [T, HIDDEN], f32, isOutput=True)

    with tile.TileContext(nc) as tc:
        from contextlib import ExitStack
        with ExitStack() as ctx:
            const = ctx.enter_context(tc.tile_pool(name="const", bufs=1))
            strips = ctx.enter_context(tc.tile_pool(name="strips", bufs=1))
            xpool = ctx.enter_context(tc.tile_pool(name="xpool", bufs=2))
            stats = ctx.enter_context(tc.tile_pool(name="stats", bufs=4))
            xnpool = ctx.enter_context(tc.tile_pool(name="xnpool", bufs=2))
            xntp = ctx.enter_context(tc.tile_pool(name="xntp", bufs=1))
            work = ctx.enter_context(tc.tile_pool(name="work", bufs=2))
            ptp = ctx.enter_context(tc.tile_pool(name="ptp", bufs=3))
            work1 = ctx.enter_context(tc.tile_pool(name="work1", bufs=1))
            obuf = ctx.enter_context(tc.tile_pool(name="obuf", bufs=2))
            psum = ctx.enter_context(tc.tile_pool(name="psum", bufs=4, space="PSUM"))
            psum2 = ctx.enter_context(tc.tile_pool(name="psum2", bufs=2, space="PSUM"))

            # constants
            w_sb = const.tile([128, KOUT, WSL], w_dt)
            nc.sync.dma_start(w_sb[:], w_d.rearrange("(o p) f -> p o f", p=128))
            wo_sb = const.tile([128, WOK, HIDDEN], mmdt)
            nc.sync.dma_start(wo_sb[:], wo_d.rearrange("(o p) f -> p o f", p=128))
            cos_sb = const.tile([128, l], f32)
            nc.sync.dma_start(cos_sb[:], cos_d[:])
            sinm_sb = const.tile([128, l], f32)
            nc.sync.dma_start(sinm_sb[:], sinm_d[:])
            mask_sb = const.tile([128, 896], f32)
            nc.sync.dma_start(mask_sb[:], mask_d[:])
            id_sb = const.tile([128, 128], f32)
            nc.sync.dma_start(id_sb[:], id_d[:])
            hb_sb = const.tile([128, WSL // 128], f32)
            nc.sync.dma_start(hb_sb[:], hb_d.rearrange("(m p) -> p m", p=128))
            eps_sb = const.tile([128, 1], f32)
            nc.vector.memset(eps_sb[:], LN_EPS)
            ones_sb = const.tile([128, 1], f32)
            nc.vector.memset(ones_sb[:], 1.0)

            # phase closures; called in an order that keeps the PE dense:
            # A0 B0 C0 A1 D0 B1 C1 D1 (A1 hides under C0/D0 on DVE/DMA)
            mu_all = const.tile([128, b * NT], f32, tag="mu_all")
            rs_all = const.tile([128, b * NT], f32, tag="rs_all")
            var_all = const.tile([128, b * NT], f32, tag="var_all")
            nmr_all = const.tile([128, b * NT], f32, tag="nmr_all")
            strips_of = {}

            def phase_a(bi):
                # LN statistics; per-tile rs so downstream can start early
                for tt in range(NT):
                    gt = bi * NT + tt
                    xt = xpool.tile([128, HIDDEN], f32, tag="xt", name="xta")
                    nc.sync.dma_start(
                        xt[:], x_d[bi * l + tt * 128: bi * l + (tt + 1) * 128, :])
                    st6 = stats.tile([128, 2, 6], f32, tag="st6")
                    nc.vector.bn_stats(st6[:, 0, :], xt[:, 0:512])
                    nc.vector.bn_stats(st6[:, 1, :], xt[:, 512:1024])
                    mv = stats.tile([128, 2], f32, tag="mv")
                    nc.vector.bn_aggr(mv[:], st6[:])
                    nc.vector.tensor_copy(mu_all[:, gt:gt + 1], mv[:, 0:1])
                    nc.scalar.activation(
                        rs_all[:, gt:gt + 1], mv[:, 1:2], AF.Sqrt, bias=eps_sb[:])
                    nc.vector.reciprocal(
                        rs_all[:, gt:gt + 1], rs_all[:, gt:gt + 1])

            def phase_b(bi):
                r0b = bi * l
                mu_s = mu_all[:, bi * NT:(bi + 1) * NT]
                rs_s = rs_all[:, bi * NT:(bi + 1) * NT]
                qT = strips.tile([128, l], mmdt, tag="qT")
                kT = strips.tile([128, l], mmdt, tag="kT")
                ffgT = strips.tile([128, EXPF, l], mmdt, tag="ffgT")
                oT = strips.tile([128, l], mmdt, tag="oT")
                vext = strips.tile([128, NT, 256], mmdt, tag="vext")
                strips_of[bi] = (qT, kT, ffgT, oT, vext)

                # v_ext ones columns (produce the softmax row sums in PV)
                nc.vector.tensor_copy(
                    vext[:, :, 64:128],
                    ones_sb[:, :, None].to_broadcast([128, NT, 64]))
                nc.vector.tensor_copy(
                    vext[:, :, 192:256],
                    ones_sb[:, :, None].to_broadcast([128, NT, 64]))

                for ci in range(NC):
                    xnT = xntp.tile([128, KOUT, 512], w_dt, tag="xnT")
                    for t4 in range(4):
                        tt = ci * 4 + t4
                        xt = xpool.tile([128, HIDDEN], f32, tag="xt", name="xtb")
                        nc.sync.dma_start(
                            xt[:], x_d[r0b + tt * 128: r0b + (tt + 1) * 128, :])
                        xn = xnpool.tile([128, HIDDEN],
                                         bf16 if opts["bf16_inproj"] else f32,
                                         tag="xn")
                        nc.vector.tensor_scalar(
                            out=xn[:], in0=xt[:],
                            scalar1=mu_s[:, tt:tt + 1], scalar2=rs_s[:, tt:tt + 1],
                            op0=OP.subtract, op1=OP.mult)
                        if opts["bf16_inproj"]:
                            for ko in range(KOUT):
                                nc.sync.dma_start(
                                    xnT[:, ko, t4 * 128:(t4 + 1) * 128],
                                    xn[:, ko * 128:(ko + 1) * 128],
                                    transpose=True)
                        else:
                            for ko in range(KOUT):
                                tfull = psum.tile([128, 512], f32, tag="p512",
                                                  name="tfull")
                                tps = tfull[:, 0:128]
                                nc.tensor.transpose(
                                    tps, xn[:, ko * 128:(ko + 1) * 128], id_sb[:])
                                if ko % 2 == 0 or not opts["act_copies"]:
                                    nc.vector.tensor_copy(
                                        xnT[:, ko, t4 * 128:(t4 + 1) * 128], tps[:])
                                else:
                                    nc.scalar.activation(
                                        xnT[:, ko, t4 * 128:(t4 + 1) * 128], tps,
                                        AF.Copy)
                    for m in range(3 + EXPF):
                        hps = psum.tile([128, 512], f32, tag="p512", name="hps")
                        for ko in range(KOUT):
                            nc.tensor.matmul(
                                hps[:],
                                w_sb[:, ko, m * 128:(m + 1) * 128],
                                xnT[:, ko, :],
                                start=(ko == 0), stop=(ko == KOUT - 1))
                        if m < 2:
                            # q/k: rope-fused eviction (+ h bias)
                            dst = (qT if m == 0 else kT)[:, ci * 512:(ci + 1) * 512]
                            cs = cos_sb[:, ci * 512:(ci + 1) * 512]
                            sn = sinm_sb[:, ci * 512:(ci + 1) * 512]
                            tmpc = work.tile([128, 512], f32, tag="rtmp1")
                            t2 = work.tile([128, 512], f32, tag="rtmp2")
                            nc.vector.scalar_tensor_tensor(
                                tmpc[:], hps[:], hb_sb[:, m:m + 1], cs,
                                OP.add, OP.mult)
                            for h in range(HPC):
                                r0 = h * 64
                                nc.vector.scalar_tensor_tensor(
                                    t2[r0:r0 + 32, :], hps[r0 + 32:r0 + 64, :],
                                    hb_sb[r0 + 32:r0 + 64, m:m + 1],
                                    sn[r0 + 32:r0 + 64, :], OP.add, OP.mult)
                                nc.vector.scalar_tensor_tensor(
                                    t2[r0 + 32:r0 + 64, :], hps[r0:r0 + 32, :],
                                    hb_sb[r0:r0 + 32, m:m + 1],
                                    sn[r0:r0 + 32, :], OP.add, OP.mult)
                            nc.vector.tensor_add(dst, tmpc[:], t2[:])
                        elif m == 2:
                            # v: bias, re-transpose to token-major
                            vtmp = work1.tile([128, 512], f32, tag="vtmp")
                            nc.vector.tensor_scalar_add(
                                vtmp[:], hps[:], hb_sb[:, 2:3])
                            for j4 in range(4):
                                jt = ci * 4 + j4
                                vfull = psum.tile([128, 512], f32, tag="p512",
                                                  name="vfull")
                                vps = vfull[:, 0:128]
                                nc.tensor.transpose(
                                    vps, vtmp[:, j4 * 128:(j4 + 1) * 128], id_sb[:])
                                if opts["act_copies"]:
                                    nc.scalar.activation(
                                        vext[:, jt, 0:64], vps[:, 0:64], AF.Copy)
                                    nc.scalar.activation(
                                        vext[:, jt, 128:192], vps[:, 64:128],
                                        AF.Copy)
                                else:
                                    nc.vector.tensor_copy(
                                        vext[:, jt, 0:64], vps[:, 0:64])
                                    nc.vector.tensor_copy(
                                        vext[:, jt, 128:192], vps[:, 64:128])
                        else:
                            nc.scalar.activation(
                                ffgT[:, m - 3, ci * 512:(ci + 1) * 512], hps[:],
                                AF.Identity if sim_gelu else AF.Gelu,
                                bias=hb_sb[:, m:m + 1])

            def phase_c(bi):
                qT, kT, ffgT, oT, vext = strips_of[bi]
                for ic in range(NC):
                    ot = [psum.tile([128, 512], f32, tag="p512",
                                    name=f"ot{_h}")
                          for _h in range(HPC)]
                    njt = (ic + 1) * 4
                    for jt in range(njt):
                        st2 = psum2.tile([128, 1024], f32, tag="st2")
                        for h in range(HPC):
                            nc.tensor.matmul(
                                st2[:, h * 512:(h + 1) * 512],
                                kT[h * 64:(h + 1) * 64, jt * 128:(jt + 1) * 128],
                                qT[h * 64:(h + 1) * 64, ic * 512:(ic + 1) * 512],
                                start=True, stop=True)
                        pt2 = ptp.tile([128, 1024], mmdt, tag="pt")
                        nc.scalar.activation(
                            pt2[:], st2[:], AF.Exp,
                            scale=float(HEAD_DIM) ** -0.5)
                        d = jt * 128 - ic * 512
                        if d >= 0:
                            eng = nc.gpsimd if opts["gp_mask"] else nc.vector
                            eng.tensor_tensor(
                                pt2[:].rearrange("p (g c) -> p g c", c=512),
                                pt2[:].rearrange("p (g c) -> p g c", c=512),
                                mask_sb[:, None, 384 - d:896 - d]
                                .to_broadcast([128, HPC, 512]),
                                OP.mult)
                        for h in range(HPC):
                            nc.tensor.matmul(
                                ot[h][:], vext[:, jt, h * 128:(h + 1) * 128],
                                pt2[:, h * 512:(h + 1) * 512],
                                start=(jt == 0), stop=(jt == njt - 1))
                    for h in range(HPC):
                        sums_sb = work1.tile([64, 512], f32, tag="sums")
                        nc.vector.reciprocal(sums_sb[:], ot[h][64:128, :])
                        nc.vector.tensor_mul(
                            oT[h * 64:(h + 1) * 64, ic * 512:(ic + 1) * 512],
                            ot[h][0:64, :], sums_sb[:])

            def phase_d(bi):
                r0b = bi * l
                qT, kT, ffgT, oT, vext = strips_of[bi]
                for tt in range(NT):
                    for n2 in range(2):
                        ops = psum.tile([128, 512], f32, tag="p512", name="ops")
                        nc.tensor.matmul(
                            ops[:], oT[:, tt * 128:(tt + 1) * 128],
                            wo_sb[:, 0, n2 * 512:(n2 + 1) * 512],
                            start=True, stop=False)
                        for kk in range(EXPF):
                            nc.tensor.matmul(
                                ops[:], ffgT[:, kk, tt * 128:(tt + 1) * 128],
                                wo_sb[:, kk + 1, n2 * 512:(n2 + 1) * 512],
                                start=False, stop=(kk == EXPF - 1))
                        ob = obuf.tile([128, 512], f32, tag="ob")
                        nc.scalar.activation(ob[:], ops[:], AF.Copy)
                        nc.sync.dma_start(
                            out_d[r0b + tt * 128: r0b + (tt + 1) * 128,
                                  n2 * 512:(n2 + 1) * 512], ob[:])

            phase_a(0)
            phase_b(0)
            phase_c(0)
            if b > 1:
                phase_a(1)
            phase_d(0)
            for bi in range(1, b):
                phase_b(bi)
                phase_c(bi)
                phase_d(bi)

    nc.compile()
    return nc


# ----------------------------------------------------------------------------
# host-side constants and per-core input slicing
# ----------------------------------------------------------------------------

def _rope_tables(l):
    inv_freq = 1.0 / (10000.0 ** (np.arange(0, HEAD_DIM, 2, dtype=np.float32)
                                  / HEAD_DIM))                       # [32]
    t = np.arange(l, dtype=np.float32)
    fr = t[None, :] * inv_freq[:, None]                              # [32, l]
    cos1 = np.cos(np.concatenate([fr, fr], axis=0))                  # [64, l]
    sin1 = np.sin(np.concatenate([fr, fr], axis=0))                  # [64, l]
    sinm1 = np.concatenate([-sin1[:32], sin1[32:]], axis=0)          # sign-folded
    # half-swapped so the stt source base partition matches the operand rows
    sinswap1 = np.concatenate([sinm1[32:], sinm1[:32]], axis=0)
    cos = np.tile(cos1, (HPC, 1)).astype(np.float32)                 # [128, l]
    sinswap = np.tile(sinswap1, (HPC, 1)).astype(np.float32)
    return cos, sinswap


def _mask_strip():
    # strip[r, u] = 1 iff u >= r + 384; diagonal block at offset d uses
    # cols [384-d : 896-d] so that mask[r, c] = (c >= r + d)
    r = np.arange(128)[:, None]
    u = np.arange(896)[None, :]
    return (u >= r + 384).astype(np.float32)


def core_inputs(x, ln_w, ln_b, W_in, W_out, c, l=L):
    """Build the per-core input map for core c (pure numpy)."""
    x = np.asarray(x, np.float32)
    ln_w = np.asarray(ln_w, np.float32)
    ln_b = np.asarray(ln_b, np.float32)
    W_in = np.asarray(W_in, np.float32)
    W_out = np.asarray(W_out, np.float32)
    T = x.shape[0] * x.shape[1] if x.ndim == 3 else x.shape[0]
    xf = np.ascontiguousarray(x.reshape(T, HIDDEN))

    qc = slice(c * QS, (c + 1) * QS)
    kc = slice(HIDDEN + c * QS, HIDDEN + (c + 1) * QS)
    vc = slice(2 * HIDDEN + c * QS, 2 * HIDDEN + (c + 1) * QS)
    fc = slice(3 * HIDDEN + c * FFS, 3 * HIDDEN + (c + 1) * FFS)
    w_raw = np.concatenate(
        [W_in[:, qc], W_in[:, kc], W_in[:, vc], W_in[:, fc]], axis=1)  # [1024, 896]
    w_slice = np.ascontiguousarray(w_raw * ln_w[:, None])
    h_bias = np.ascontiguousarray(ln_b @ w_raw)                        # [896]
    wo_slice = np.ascontiguousarray(np.concatenate(
        [W_out[c * QS:(c + 1) * QS, :],
         W_out[HIDDEN + c * FFS: HIDDEN + (c + 1) * FFS, :]], axis=0))  # [640, 1024]

    cos, sinm = _rope_tables(l)
    return {
        "x": xf,
        "w_in": w_slice,
        "w_out": wo_slice,
        "h_bias": h_bias,
        "cos_t": cos,
        "sinm_t": sinm,
        "mask_t": _mask_strip(),
        "ident": np.eye(128, dtype=np.float32),
    }


# ----------------------------------------------------------------------------
# entry point
# ----------------------------------------------------------------------------

_PROG_CACHE = {}


def kernel(x, ln_w, ln_b, W_in, W_out):
    global LAST_RESULTS
    from concourse import bass_utils
    from concourse.bass_interp import get_hw_module

    x = np.asarray(x, np.float32)
    b, l = x.shape[0], x.shape[1]

    import os as _os
    opts = {}
    for k in ("gp_mask", "act_copies", "approx_recip", "bf16_inproj"):
        v = _os.environ.get("BASS_OPT_" + k.upper())
        if v is not None:
            opts[k] = bool(int(v))
    key = (b, l, tuple(sorted(opts.items())))
    if key not in _PROG_CACHE:
        _PROG_CACHE[key] = build_program(b=b, l=l, use_fp32r=True, debug=False,
                                         opts=opts)
    nc = _PROG_CACHE[key]

    in_maps = [core_inputs(x, ln_w, ln_b, W_in, W_out, c, l=l)
               for c in range(NCORES)]
    if opts.get("bf16_inproj"):
        import ml_dtypes
        for m in in_maps:
            m["w_in"] = np.ascontiguousarray(
                m["w_in"].astype(ml_dtypes.bfloat16))

    old_m = nc.m
    nc.m = get_hw_module(nc.m)
    try:
        res = bass_utils.run_bass_kernel_spmd(
            nc, in_maps, core_ids=list(range(NCORES)),
            trace=bool(int(__import__("os").environ.get("BASS_TRACE_RUN", "0"))))
    finally:
        nc.m = old_m
    LAST_RESULTS = res

    acc = np.zeros((b * l, HIDDEN), np.float64)
    for r in res.results:
        acc += r["out"].astype(np.float64)
    return acc.reshape(b, l, HIDDEN).astype(np.float32)

